# revision 2
# baseline (speedup 1.0000x reference)
"""HGCN 2-layer GNN message passing kernel for 8 Trainium2 NeuronCores.

Math notes (vs the reference):
  - alpha = softmax over a size-1 axis == 1.0 exactly, so the attention
    branch (Wa, ba, leaky_relu, softmax) contributes nothing.
  - msg = x_j * (-|curv|), so each layer is
        out = segment_sum((x @ W + b)[src], dst) * s      with s = -|curv|
    and since matmul distributes over the segment sum:
        out = segment_sum(x[src], dst) @ (W*s) + deg * (b*s)
    i.e. aggregate raw features first, apply the (scaled) linear after.
  - layer1: h = relu(out1); layer2: log_softmax(out2).

Sharding: nodes are range-partitioned across the 8 cores by destination
(6250 nodes each).  Each core processes the edges whose dst lands in its
range (edge counts are ~equal for random graphs).  Edges are sorted by dst
on the host; per 128-node dst block the core gathers x[src] rows with
dma_gather and segment-sums them with one-hot matmuls accumulated in PSUM.
Between layers, the per-core y2 = h @ W2f slices are AllGathered so every
core can gather layer-2 messages from the full table.
"""

import os
import sys

import numpy as np

if "/opt/trn_rl_repo" not in sys.path:
    sys.path.insert(0, "/opt/trn_rl_repo")

import concourse.bacc as bacc
import concourse.bass as bass
import concourse.mybir as mybir
import concourse.tile as tile
from concourse.bass_utils import run_bass_kernel_spmd
from concourse.masks import make_identity

P = 128
N_CORES = 8
SPLIT = 32768  # int16 index limit for dma_gather


# ---------------------------------------------------------------------------
# host-side edge preprocessing
# ---------------------------------------------------------------------------

def _wrap_idx(raw):
    """[n*128] row indices -> [128, n*8] int16 dma_gather index layout
    (wrapped in 16 partitions, replicated across the 8 gpsimd cores)."""
    n = raw.shape[0]
    w = raw.reshape(n // 16, 16).T.astype(np.int16)  # [16, n//16]
    return np.tile(w, (8, 1))


def _preprocess(edge_index, n_nodes, n_cores):
    """Sort edges (plus self loops) by dst, split per core / per 128-dst
    block / by src<SPLIT, and build the padded per-core index arrays."""
    src = np.concatenate([edge_index[0], np.arange(n_nodes, dtype=np.int64)])
    dst = np.concatenate([edge_index[1], np.arange(n_nodes, dtype=np.int64)])
    order = np.argsort(dst, kind="stable")
    src_s = src[order].astype(np.int64)
    dst_s = dst[order].astype(np.int64)
    deg = np.bincount(dst, minlength=n_nodes).astype(np.float32)

    npc = n_nodes // n_cores  # nodes per core
    nblk = (npc + P - 1) // P
    # per (core, block): low/high src arrays and dst_local arrays
    lows, highs, dlows, dhighs = {}, {}, {}, {}
    cl = np.zeros((n_cores, nblk), dtype=np.int64)
    ch = np.zeros((n_cores, nblk), dtype=np.int64)
    for c in range(n_cores):
        for b in range(nblk):
            lo = c * npc + b * P
            hi = min(lo + P, (c + 1) * npc)
            e0 = np.searchsorted(dst_s, lo, side="left")
            e1 = np.searchsorted(dst_s, hi, side="left")
            s = src_s[e0:e1]
            dl = (dst_s[e0:e1] - lo).astype(np.float32)
            m = s < SPLIT
            lows[c, b], dlows[c, b] = s[m], dl[m]
            highs[c, b], dhighs[c, b] = s[~m] - SPLIT, dl[~m]
            cl[c, b] = (len(lows[c, b]) + P - 1) // P
            ch[c, b] = (len(highs[c, b]) + P - 1) // P
    CL = cl.max(axis=0)  # shared program: max chunks per block position
    CH = ch.max(axis=0)
    CL = np.maximum(CL, 1)
    CH = np.maximum(CH, 1)
    totc = int(CL.sum() + CH.sum())

    idx_arrs, dloc_arrs = [], []
    for c in range(n_cores):
        idx_np = np.zeros((P, 8 * totc), dtype=np.int16)
        dloc_np = np.full((P, totc), float(P), dtype=np.float32)
        col = 0
        for b in range(nblk):
            for arrs, darrs, nch in ((lows, dlows, CL[b]), (highs, dhighs, CH[b])):
                a = arrs[c, b]
                d = darrs[c, b]
                ni = int(nch) * P
                pad_a = np.zeros(ni, dtype=np.int64)
                pad_a[: len(a)] = a
                pad_d = np.full(ni, float(P), dtype=np.float32)
                pad_d[: len(d)] = d
                idx_np[:, 8 * col : 8 * (col + int(nch))] = _wrap_idx(pad_a)
                dloc_np[:, col : col + int(nch)] = pad_d.reshape(int(nch), P).T
                col += int(nch)
        idx_arrs.append(idx_np)
        dloc_arrs.append(dloc_np)
    return CL, CH, idx_arrs, dloc_arrs, deg


# ---------------------------------------------------------------------------
# device program
# ---------------------------------------------------------------------------

def _build_program(CL, CH, n_nodes, d_in, d_hid, d_out, n_cores):
    npc = n_nodes // n_cores
    nblk = len(CL)
    totc = int(CL.sum() + CH.sum())
    cmax = int((CL + CH).max())
    f32 = mybir.dt.float32

    nc = bacc.Bacc(
        "TRN2",
        target_bir_lowering=False,
        debug=False,
        num_devices=n_cores,
        num_swdge_queues=4,
    )
    x_ap = nc.dram_tensor("x", [n_nodes, d_in], f32, kind="ExternalInput").ap()
    w1_ap = nc.dram_tensor("w1", [d_in, d_hid], f32, kind="ExternalInput").ap()
    w2_ap = nc.dram_tensor("w2", [d_hid, d_out], f32, kind="ExternalInput").ap()
    idx_ap = nc.dram_tensor("idx", [P, 8 * totc], mybir.dt.int16, kind="ExternalInput").ap()
    dloc_ap = nc.dram_tensor("dloc", [P, totc], f32, kind="ExternalInput").ap()
    degb1_ap = nc.dram_tensor("degb1", [npc, d_hid], f32, kind="ExternalInput").ap()
    degb2_ap = nc.dram_tensor("degb2", [npc, d_out], f32, kind="ExternalInput").ap()
    out_ap = nc.dram_tensor("out", [npc, d_out], f32, kind="ExternalOutput").ap()

    gq = [0]  # rotating swdge queue assignment

    def gather(gt, table, idx_sb, col, nch, elem):
        ni = int(nch) * P
        nc.gpsimd.dma_gather(
            out_ap=gt.rearrange("p (c e) -> p c e", e=elem),
            in_ap=table,
            idxs_ap=idx_sb[:, 8 * col : 8 * (col + int(nch))],
            num_idxs=ni,
            num_idxs_reg=ni,
            elem_size=elem,
            single_packet=False,
            queue_num=gq[0] % 4,
        )
        gq[0] += 1

    with tile.TileContext(nc) as tc:
        with (
            tc.tile_pool(name="const", bufs=1) as cp,
            tc.tile_pool(name="gx", bufs=3) as gxp,
            tc.tile_pool(name="oh", bufs=3) as ohp,
            tc.tile_pool(name="blk", bufs=3) as bp,
            tc.tile_pool(name="dram", bufs=1, space="DRAM") as dram,
        ):
            w1_sb = cp.tile([d_in, d_hid], f32)
            w2_sb = cp.tile([d_hid, d_out], f32)
            ident = cp.tile([P, P], f32)
            idx_sb = cp.tile([P, 8 * totc], mybir.dt.int16)
            dloc_sb = cp.tile([P, totc], f32)
            iota_i = cp.tile([P, cmax * P], mybir.dt.int32)
            iota_f = cp.tile([P, cmax * P], f32)
            nc.sync.dma_start(out=w1_sb[:], in_=w1_ap[:])
            nc.sync.dma_start(out=w2_sb[:], in_=w2_ap[:])
            nc.sync.dma_start(out=idx_sb[:], in_=idx_ap[:])
            nc.sync.dma_start(out=dloc_sb[:], in_=dloc_ap[:])
            make_identity(nc, ident[:])
            nc.gpsimd.iota(
                iota_i[:].rearrange("p (c m) -> p c m", m=P),
                pattern=[[0, cmax], [1, P]],
                base=0,
                channel_multiplier=0,
            )
            nc.vector.tensor_copy(out=iota_f[:], in_=iota_i[:])

            y2slice = dram.tile([npc, d_out], f32)
            y2full = dram.tile([n_nodes, d_out], f32, addr_space="Shared")

            def onehot(col, nch):
                """one-hot [128 edges, (nch*P) cols->128 nodes] per chunk."""
                oh = ohp.tile([P, cmax * P], f32, tag="oh")
                n = int(nch)
                nc.vector.tensor_tensor(
                    out=oh[:, : n * P].rearrange("p (c m) -> p c m", m=P),
                    in0=dloc_sb[:, col : col + n, None].to_broadcast([P, n, P]),
                    in1=iota_f[:, : n * P].rearrange("p (c m) -> p c m", m=P),
                    op=mybir.AluOpType.is_equal,
                )
                return oh

            # ---------------- phase 1: layer 1 + y2 slices ----------------
            with (
                tc.tile_pool(name="psA", bufs=2, space="PSUM") as psA,
                tc.tile_pool(name="psH", bufs=2, space="PSUM") as psH,
                tc.tile_pool(name="psT", bufs=2, space="PSUM") as psT,
                tc.tile_pool(name="psY", bufs=2, space="PSUM") as psY,
            ):
                col = 0
                for b in range(nblk):
                    nbsz = min(P, npc - b * P)
                    ctot = int(CL[b] + CH[b])
                    gx = gxp.tile([P, cmax * d_in], f32, tag="gx")
                    gather(gx[:, : int(CL[b]) * d_in], x_ap[:SPLIT, :], idx_sb, col, CL[b], d_in)
                    gather(
                        gx[:, int(CL[b]) * d_in : ctot * d_in],
                        x_ap[SPLIT:, :],
                        idx_sb,
                        col + int(CL[b]),
                        CH[b],
                        d_in,
                    )
                    oh = onehot(col, ctot)
                    aggT = psA.tile([P, P], f32, space="PSUM", tag="aggT")
                    for k in range(ctot):
                        # aggT[feat, node] += gx_chunk.T @ oh_chunk
                        nc.tensor.matmul(
                            out=aggT[:d_in, :],
                            lhsT=gx[:, k * d_in : (k + 1) * d_in],
                            rhs=oh[:, k * P : (k + 1) * P],
                            start=(k == 0),
                            stop=(k == ctot - 1),
                        )
                    aggT_sb = bp.tile([P, P], f32, tag="aggT_sb")
                    nc.vector.tensor_copy(out=aggT_sb[:d_in, :], in_=aggT[:d_in, :])
                    # h = aggX @ W1f + deg*b1f ; relu
                    h_ps = psH.tile([P, d_hid], f32, space="PSUM", tag="h")
                    nc.tensor.matmul(
                        out=h_ps[:], lhsT=aggT_sb[:d_in, :], rhs=w1_sb[:], start=True, stop=True
                    )
                    db1 = bp.tile([P, d_hid], f32, tag="db1")
                    nc.sync.dma_start(
                        out=db1[:nbsz, :], in_=degb1_ap[b * P : b * P + nbsz, :]
                    )
                    h_sum = bp.tile([P, d_hid], f32, tag="h_sum")
                    nc.vector.tensor_add(
                        out=h_sum[:nbsz, :], in0=h_ps[:nbsz, :], in1=db1[:nbsz, :]
                    )
                    h_sb = bp.tile([P, d_hid], f32, tag="h_sb")
                    nc.scalar.activation(
                        out=h_sb[:nbsz, :], in_=h_sum[:nbsz, :],
                        func=mybir.ActivationFunctionType.Relu,
                    )
                    # y2 = h @ W2f  (row-major into the AllGather slice)
                    hT_ps = psT.tile([P, P], f32, space="PSUM", tag="hT")
                    nc.tensor.transpose(
                        out=hT_ps[:, :nbsz], in_=h_sb[:nbsz, :], identity=ident[:nbsz, :nbsz]
                    )
                    hT_sb = bp.tile([P, P], f32, tag="hT_sb")
                    nc.vector.tensor_copy(out=hT_sb[:, :nbsz], in_=hT_ps[:, :nbsz])
                    y2_ps = psY.tile([P, d_out], f32, space="PSUM", tag="y2")
                    nc.tensor.matmul(
                        out=y2_ps[:nbsz, :], lhsT=hT_sb[:, :nbsz], rhs=w2_sb[:], start=True, stop=True
                    )
                    y2_sb = bp.tile([P, d_out], f32, tag="y2_sb")
                    nc.vector.tensor_copy(out=y2_sb[:nbsz, :], in_=y2_ps[:nbsz, :])
                    nc.sync.dma_start(
                        out=y2slice[b * P : b * P + nbsz, :], in_=y2_sb[:nbsz, :]
                    )
                    col += ctot

            # ---------------- exchange ----------------
            nc.gpsimd.collective_compute(
                "AllGather",
                mybir.AluOpType.bypass,
                replica_groups=[list(range(n_cores))],
                ins=[y2slice[:].opt()],
                outs=[y2full[:].opt()],
            )

            # ---------------- phase 2: layer 2 + log_softmax ----------------
            with tc.tile_pool(name="psO", bufs=2, space="PSUM") as psO:
                col = 0
                for b in range(nblk):
                    nbsz = min(P, npc - b * P)
                    ctot = int(CL[b] + CH[b])
                    gy = gxp.tile([P, cmax * d_out], f32, tag="gy")
                    gather(gy[:, : int(CL[b]) * d_out], y2full[:SPLIT, :], idx_sb, col, CL[b], d_out)
                    gather(
                        gy[:, int(CL[b]) * d_out : ctot * d_out],
                        y2full[SPLIT:, :],
                        idx_sb,
                        col + int(CL[b]),
                        CH[b],
                        d_out,
                    )
                    oh = onehot(col, ctot)
                    o_ps = psO.tile([P, d_out], f32, space="PSUM", tag="o")
                    for k in range(ctot):
                        # out2[node, feat] += oh_chunk.T @ gy_chunk
                        nc.tensor.matmul(
                            out=o_ps[:],
                            lhsT=oh[:, k * P : (k + 1) * P],
                            rhs=gy[:, k * d_out : (k + 1) * d_out],
                            start=(k == 0),
                            stop=(k == ctot - 1),
                        )
                    db2 = bp.tile([P, d_out], f32, tag="db2")
                    nc.sync.dma_start(
                        out=db2[:nbsz, :], in_=degb2_ap[b * P : b * P + nbsz, :]
                    )
                    t_sb = bp.tile([P, d_out], f32, tag="t_sb")
                    nc.vector.tensor_add(
                        out=t_sb[:nbsz, :], in0=o_ps[:nbsz, :], in1=db2[:nbsz, :]
                    )
                    # log_softmax over the free axis
                    mx = bp.tile([P, 1], f32, tag="mx")
                    nc.vector.reduce_max(out=mx[:nbsz, :], in_=t_sb[:nbsz, :], axis=mybir.AxisListType.X)
                    tm = bp.tile([P, d_out], f32, tag="tm")
                    nc.vector.tensor_scalar_sub(out=tm[:nbsz, :], in0=t_sb[:nbsz, :], scalar1=mx[:nbsz, :])
                    ex = bp.tile([P, d_out], f32, tag="ex")
                    nc.scalar.activation(
                        out=ex[:nbsz, :], in_=tm[:nbsz, :], func=mybir.ActivationFunctionType.Exp
                    )
                    sm = bp.tile([P, 1], f32, tag="sm")
                    nc.vector.reduce_sum(out=sm[:nbsz, :], in_=ex[:nbsz, :], axis=mybir.AxisListType.X)
                    ls = bp.tile([P, 1], f32, tag="ls")
                    nc.scalar.activation(
                        out=ls[:nbsz, :], in_=sm[:nbsz, :], func=mybir.ActivationFunctionType.Ln
                    )
                    res = bp.tile([P, d_out], f32, tag="res")
                    nc.vector.tensor_scalar_sub(out=res[:nbsz, :], in0=tm[:nbsz, :], scalar1=ls[:nbsz, :])
                    nc.sync.dma_start(
                        out=out_ap[b * P : b * P + nbsz, :], in_=res[:nbsz, :]
                    )
                    col += ctot

    nc.compile()
    return nc


_PROGRAM_CACHE = {}


def _run(x, edge_index, W1f, b1f, W2f, b2f, n_cores=N_CORES):
    n_nodes, d_in = x.shape
    d_hid = W1f.shape[1]
    d_out = W2f.shape[1]
    npc = n_nodes // n_cores

    CL, CH, idx_arrs, dloc_arrs, deg = _preprocess(edge_index, n_nodes, n_cores)

    key = (n_nodes, d_in, d_hid, d_out, n_cores, tuple(CL), tuple(CH))
    if key not in _PROGRAM_CACHE:
        _PROGRAM_CACHE[key] = _build_program(CL, CH, n_nodes, d_in, d_hid, d_out, n_cores)
    nc = _PROGRAM_CACHE[key]

    in_maps = []
    for c in range(n_cores):
        deg_c = deg[c * npc : (c + 1) * npc]
        in_maps.append(
            {
                "x": np.ascontiguousarray(x),
                "w1": np.ascontiguousarray(W1f),
                "w2": np.ascontiguousarray(W2f),
                "idx": idx_arrs[c],
                "dloc": dloc_arrs[c],
                "degb1": np.ascontiguousarray(np.outer(deg_c, b1f).astype(np.float32)),
                "degb2": np.ascontiguousarray(np.outer(deg_c, b2f).astype(np.float32)),
            }
        )
    res = run_bass_kernel_spmd(
        nc,
        in_maps,
        core_ids=list(range(n_cores)),
        trace=bool(os.environ.get("KERNEL_TRACE")),
    )
    out = np.concatenate([res.results[c]["out"] for c in range(n_cores)], axis=0)
    return out, res


def kernel(x, edge_index, W1, b1, Wa1, ba1, curv1, W2, b2, Wa2, ba2, curv2):
    x = np.asarray(x, dtype=np.float32)
    edge_index = np.asarray(edge_index).astype(np.int64)
    s1 = -abs(float(np.asarray(curv1).reshape(-1)[0]))
    s2 = -abs(float(np.asarray(curv2).reshape(-1)[0]))
    W1f = np.asarray(W1, dtype=np.float32) * s1
    b1f = np.asarray(b1, dtype=np.float32) * s1
    W2f = np.asarray(W2, dtype=np.float32) * s2
    b2f = np.asarray(b2, dtype=np.float32) * s2
    out, _ = _run(x, edge_index, W1f, b1f, W2f, b2f)
    return out
